# revision 18
# baseline (speedup 1.0000x reference)
"""Morphological dilation (7x7 additive SE, zero 'same' padding) on 8 trn2 cores.

out[b,c,i,j] = max_{a,t} ( xpad[b,c,i+a,j+t] + w[a,t] ),  x: (8,8,512,512) f32.

Sharding: pure data parallel - 64 images (B*C) split 8 per core; the 7x7
weight is replicated. No cross-core communication.

Per-core plan (both variants):
 1. Build a zero-padded copy of each image in DRAM scratch: xpad[g] is
    518x518 with the image at [3:515, 3:515] (borders DMA-zeroed from an
    SBUF zero tile). Every later load is then a single in-bounds DMA: no
    edge slivers, no per-tile memsets. Fills are interleaved with compute
    (two images of lookahead) so the DMA queues never stall the DVE.
 2. Per image ("slab"): for each vertical tap a in 0..6, load the
    row-shifted slab sh_a = xpad[g, a:a+512, :] into SBUF as [128, 4, 518]
    (partition = row within 128-row strip, free = strip x padded row).
    Horizontal taps t are free-dim offsets of the same tile.

f32 variant: 49 fused add+max scalar_tensor_tensor ops per image on the
vector engine (1 elem/cycle/lane), interleaved over 2 accumulator chains.

f16 variant (default): data cast to fp16 once (DVE copy; xpad is fp16).
Per tap: temp = sh + w[a,t] (tensor_scalar, 4x mode on DVE for 4B-aligned
even t; ACT Identity with per-partition bias AP otherwise - ACT is
alignment/dtype independent), then acc = max(acc, temp) (tensor_tensor,
2x mode). ~35 of 49 adds go to ACT so both engines finish together.
4 accumulator chains; fp16 accs converted to fp32 on the way out.
"""

import os
import sys

for p in ("/root/.axon_site", "/root/.axon_site/_ro/trn_rl_repo",
          "/root/.axon_site/_ro/pypackages", "/opt/trn_rl_repo"):
    if os.path.isdir(p) and p not in sys.path:
        sys.path.append(p)

import numpy as np

import concourse.bass as bass
import concourse.bacc as bacc
import concourse.mybir as mybir
from concourse.bass_utils import run_bass_kernel_spmd
from concourse.tile import TileContext

KH = KW = 7
PAD = 3
H = W = 512
N_CORES = 8
IMGS_PER_CORE = 8  # 8*8 = 64 images total
WPAD = W + 2 * PAD  # 518
S = H // 128  # 4 strips of 128 rows per image

f32 = mybir.dt.float32
f16 = mybir.dt.float16
ADD = mybir.AluOpType.add
MAX = mybir.AluOpType.max
IDENT = mybir.ActivationFunctionType.Identity

VARIANT = os.environ.get("BASS_DILATE_VARIANT", "f16")
# taps whose add runs on ACT (odd t must: fp16 4x tensor_scalar needs 4B
# alignment; odd-t slices are only 2B aligned). Then pad with even-t taps
# until ACT and DVE are balanced (~35 ACT adds per image).
N_ACT_EXTRA = int(os.environ.get("BASS_DILATE_ACT_EXTRA", "13"))


def _emit_pad_fill(nc, xpad, zt, g):
    """Zero xpad[g] (interior gets overwritten by the image afterwards)."""
    for r0 in range(0, WPAD, 128):
        r1 = min(WPAD, r0 + 128)
        nc.sync.dma_start(out=xpad[g, r0:r1, :], in_=zt[0:r1 - r0, :])


def _build_f32():
    nc = bacc.Bacc("TRN2")
    x = nc.dram_tensor("x", (IMGS_PER_CORE, H, W), f32, kind="ExternalInput")
    wt = nc.dram_tensor("weight", (KH, KW), f32, kind="ExternalInput")
    out = nc.dram_tensor("out", (IMGS_PER_CORE, H, W), f32, kind="ExternalOutput")
    NCH = 2

    with TileContext(nc) as tc:
        with (
            tc.tile_pool(name="const", bufs=1) as cpool,
            tc.tile_pool(name="dram", bufs=1, space="DRAM") as dpool,
            tc.tile_pool(name="sh", bufs=12) as shpool,
            tc.tile_pool(name="acc", bufs=2) as apool,
        ):
            w_sb = cpool.tile([128, KH * KW], f32)
            nc.sync.dma_start(
                out=w_sb[:, :],
                in_=wt[:, :].rearrange("a b -> (a b)").unsqueeze(0)
                .broadcast_to([128, KH * KW]),
            )
            zt = cpool.tile([128, WPAD], f32)
            nc.vector.memset(zt[:, :], 0.0)

            xpad = dpool.tile([IMGS_PER_CORE, WPAD, WPAD], f32)

            def fill(g):
                _emit_pad_fill(nc, xpad, zt, g)
                nc.sync.dma_start(
                    out=xpad[g, PAD:PAD + H, PAD:PAD + W], in_=x[g, :, :]
                )

            fill(0)
            fill(1)
            for g in range(IMGS_PER_CORE):
                accs = [
                    apool.tile([128, S, W], f32, tag=f"acc{c}", name=f"acc{c}_{g}")
                    for c in range(NCH)
                ]
                acc_used = [False] * NCH
                for a in range(KH):
                    sh = shpool.tile([128, S, WPAD], f32, tag="sh", name=f"sh_{g}_{a}")
                    nc.sync.dma_start(
                        out=sh[:, :, :],
                        in_=xpad[g, a:a + H, :].rearrange("(s p) w -> p s w", p=128),
                    )
                    for t in range(KW):
                        k = a * KW + t
                        c = k % NCH
                        in0 = sh[:, :, t:t + W]
                        if not acc_used[c]:
                            nc.vector.tensor_scalar(
                                out=accs[c][:, :, :], in0=in0,
                                scalar1=w_sb[:, k:k + 1], scalar2=None, op0=ADD,
                            )
                            acc_used[c] = True
                        else:
                            nc.vector.scalar_tensor_tensor(
                                out=accs[c][:, :, :], in0=in0,
                                scalar=w_sb[:, k:k + 1], in1=accs[c][:, :, :],
                                op0=ADD, op1=MAX,
                            )
                for c in range(1, NCH):
                    nc.vector.tensor_tensor(
                        out=accs[0][:, :, :], in0=accs[0][:, :, :],
                        in1=accs[c][:, :, :], op=MAX,
                    )
                nc.sync.dma_start(
                    out=out[g].rearrange("(s p) w -> p s w", p=128),
                    in_=accs[0][:, :, :],
                )
                if g + 2 < IMGS_PER_CORE:
                    fill(g + 2)
    nc.finalize()
    return nc


def _build_f16():
    nc = bacc.Bacc("TRN2")
    x = nc.dram_tensor("x", (IMGS_PER_CORE, H, W), f32, kind="ExternalInput")
    wt = nc.dram_tensor("weight", (KH, KW), f32, kind="ExternalInput")
    out = nc.dram_tensor("out", (IMGS_PER_CORE, H, W), f32, kind="ExternalOutput")
    NCH = int(os.environ.get("BASS_DILATE_NCH", "4"))
    N_DIRECT = int(os.environ.get("BASS_DILATE_DIRECT", "0"))

    # adds on ACT: all odd t (alignment), plus N_ACT_EXTRA even-t for balance
    act_taps = {(a, t) for a in range(KH) for t in range(KW) if t % 2 == 1}
    even_taps = [(a, t) for a in range(KH) for t in range(KW) if t % 2 == 0]
    step = max(1, len(even_taps) // max(1, N_ACT_EXTRA))
    for i in range(0, min(N_ACT_EXTRA, len(even_taps))):
        act_taps.add(even_taps[(i * step) % len(even_taps)])

    with TileContext(nc) as tc:
        with (
            tc.tile_pool(name="const", bufs=1) as cpool,
            tc.tile_pool(name="dram", bufs=1, space="DRAM") as dpool,
            tc.tile_pool(name="sh", bufs=12) as shpool,
            tc.tile_pool(name="tmp", bufs=8) as tpool,
            tc.tile_pool(name="acc", bufs=2) as apool,
        ):
            w_sb = cpool.tile([128, KH * KW], f32)
            nc.sync.dma_start(
                out=w_sb[:, :],
                in_=wt[:, :].rearrange("a b -> (a b)").unsqueeze(0)
                .broadcast_to([128, KH * KW]),
            )
            zt = cpool.tile([128, WPAD], f16)
            nc.vector.memset(zt[:, :], 0.0)

            xpad = dpool.tile([IMGS_PER_CORE, WPAD, WPAD], f16)
            zdram = dpool.tile([WPAD, WPAD], f16)
            for r0 in range(0, WPAD, 128):
                r1 = min(WPAD, r0 + 128)
                nc.sync.dma_start(out=zdram[r0:r1, :], in_=zt[0:r1 - r0, :])

            def fill(g):
                nc.sync.dma_start(out=xpad[g, :, :], in_=zdram[:, :])
                # fp32 -> fp16 cast happens inside the SWDGE DMA
                nc.gpsimd.dma_start(
                    out=xpad[g, PAD:PAD + H, PAD:PAD + W], in_=x[g, :, :]
                )

            def load_sh_direct(sh, g, a):
                """Image g's shifted slab straight from x (SWDGE casts
                f32->f16); pad columns via DVE memset, pad rows from zt.
                No input dependencies, so the pipeline starts immediately."""
                d = a - PAD
                nc.vector.memset(sh[:, :, 0:PAD], 0.0)
                nc.vector.memset(sh[:, :, PAD + W:WPAD], 0.0)
                if d < 0:
                    nc.gpsimd.dma_start(out=sh[-d:128, 0, PAD:PAD + W],
                                        in_=x[g, 0:128 + d, :])
                    nc.gpsimd.dma_start(
                        out=sh[:, 1:S, PAD:PAD + W],
                        in_=x[g, 128 + d:(S - 1) * 128 + 128 + d, :]
                        .rearrange("(s p) w -> p s w", p=128),
                    )
                    nc.gpsimd.dma_start(out=sh[0:-d, 0, :], in_=zt[0:-d, :])
                elif d == 0:
                    nc.gpsimd.dma_start(
                        out=sh[:, :, PAD:PAD + W],
                        in_=x[g].rearrange("(s p) w -> p s w", p=128),
                    )
                else:
                    nc.gpsimd.dma_start(
                        out=sh[:, 0:S - 1, PAD:PAD + W],
                        in_=x[g, d:(S - 1) * 128 + d, :]
                        .rearrange("(s p) w -> p s w", p=128),
                    )
                    nc.gpsimd.dma_start(out=sh[0:128 - d, S - 1, PAD:PAD + W],
                                        in_=x[g, (S - 1) * 128 + d:H, :])
                    nc.gpsimd.dma_start(out=sh[128 - d:128, S - 1, :],
                                        in_=zt[0:d, :])

            for g in range(N_DIRECT, N_DIRECT + 2):
                if g < IMGS_PER_CORE:
                    fill(g)
            for g in range(IMGS_PER_CORE):
                accs = [
                    apool.tile([128, S, W], f16, tag=f"acc{c}", name=f"acc{c}_{g}")
                    for c in range(NCH)
                ]
                acc_used = [False] * NCH
                for a in range(KH):
                    sh = shpool.tile([128, S, WPAD], f16, tag="sh", name=f"sh_{g}_{a}")
                    if g < N_DIRECT:
                        load_sh_direct(sh, g, a)
                    else:
                        nc.sync.dma_start(
                            out=sh[:, :, :],
                            in_=xpad[g, a:a + H, :]
                            .rearrange("(s p) w -> p s w", p=128),
                        )
                    for t in range(KW):
                        k = a * KW + t
                        c = k % NCH
                        in0 = sh[:, :, t:t + W]
                        if not acc_used[c]:
                            dst = accs[c][:, :, :]
                        else:
                            tmp = tpool.tile([128, S, W], f16, tag="tmp",
                                             name=f"tmp_{g}_{k}")
                            dst = tmp[:, :, :]
                        if (a, t) in act_taps:
                            nc.scalar.activation(
                                out=dst, in_=in0, func=IDENT,
                                bias=w_sb[:, k:k + 1], scale=1.0,
                            )
                        else:
                            nc.vector.tensor_scalar(
                                out=dst, in0=in0,
                                scalar1=w_sb[:, k:k + 1], scalar2=None, op0=ADD,
                            )
                        if acc_used[c]:
                            nc.vector.tensor_tensor(
                                out=accs[c][:, :, :], in0=accs[c][:, :, :],
                                in1=dst, op=MAX,
                            )
                        acc_used[c] = True
                for c in range(1, NCH):
                    nc.vector.tensor_tensor(
                        out=accs[0][:, :, :], in0=accs[0][:, :, :],
                        in1=accs[c][:, :, :], op=MAX,
                    )
                nc.gpsimd.dma_start(
                    out=out[g].rearrange("(s p) w -> p s w", p=128),
                    in_=accs[0][:, :, :],
                )
                if g + N_DIRECT + 2 < IMGS_PER_CORE:
                    fill(g + N_DIRECT + 2)
    nc.finalize()
    return nc


_NC_CACHE = {}


def _get_nc(variant=None):
    variant = variant or VARIANT
    if variant not in _NC_CACHE:
        _NC_CACHE[variant] = _build_f16() if variant == "f16" else _build_f32()
    return _NC_CACHE[variant]


def _run(x, weight, trace=False, variant=None, trace_kwargs=None):
    x = np.ascontiguousarray(x, dtype=np.float32)
    weight = np.ascontiguousarray(weight, dtype=np.float32)
    B, C, Hx, Wx = x.shape
    xs = x.reshape(B * C, Hx, Wx)
    per = (B * C) // N_CORES
    in_maps = [
        {"x": np.ascontiguousarray(xs[i * per:(i + 1) * per]), "weight": weight}
        for i in range(N_CORES)
    ]
    nc = _get_nc(variant)
    res = run_bass_kernel_spmd(
        nc, in_maps, list(range(N_CORES)),
        trace=trace, trace_cores=[0] if trace else None,
        **(trace_kwargs or {}),
    )
    outs = np.concatenate([res.results[i]["out"] for i in range(N_CORES)], axis=0)
    return outs.reshape(B, C, Hx, Wx), res


def kernel(x, weight):
    out, _ = _run(x, weight)
    return out


# revision 19
# speedup vs baseline: 1.0264x; 1.0264x over previous
"""Morphological dilation (7x7 additive SE, zero 'same' padding) on 8 trn2 cores.

out[b,c,i,j] = max_{a,t} ( xpad[b,c,i+a,j+t] + w[a,t] ),  x: (8,8,512,512) f32.

Sharding: pure data parallel - 64 images (B*C) split 8 per core; the 7x7
weight is replicated. No cross-core communication.

Per-core plan (both variants):
 1. Build a zero-padded copy of each image in DRAM scratch: xpad[g] is
    518x518 with the image at [3:515, 3:515] (borders DMA-zeroed from an
    SBUF zero tile). Every later load is then a single in-bounds DMA: no
    edge slivers, no per-tile memsets. Fills are interleaved with compute
    (two images of lookahead) so the DMA queues never stall the DVE.
 2. Per image ("slab"): for each vertical tap a in 0..6, load the
    row-shifted slab sh_a = xpad[g, a:a+512, :] into SBUF as [128, 4, 518]
    (partition = row within 128-row strip, free = strip x padded row).
    Horizontal taps t are free-dim offsets of the same tile.

f32 variant: 49 fused add+max scalar_tensor_tensor ops per image on the
vector engine (1 elem/cycle/lane), interleaved over 2 accumulator chains.

f16 variant (default): data cast to fp16 once (DVE copy; xpad is fp16).
Per tap: temp = sh + w[a,t] (tensor_scalar, 4x mode on DVE for 4B-aligned
even t; ACT Identity with per-partition bias AP otherwise - ACT is
alignment/dtype independent), then acc = max(acc, temp) (tensor_tensor,
2x mode). ~35 of 49 adds go to ACT so both engines finish together.
4 accumulator chains; fp16 accs converted to fp32 on the way out.
"""

import os
import sys

for p in ("/root/.axon_site", "/root/.axon_site/_ro/trn_rl_repo",
          "/root/.axon_site/_ro/pypackages", "/opt/trn_rl_repo"):
    if os.path.isdir(p) and p not in sys.path:
        sys.path.append(p)

import numpy as np

import concourse.bass as bass
import concourse.bacc as bacc
import concourse.mybir as mybir
from concourse.bass_utils import run_bass_kernel_spmd
from concourse.tile import TileContext

KH = KW = 7
PAD = 3
H = W = 512
N_CORES = 8
IMGS_PER_CORE = 8  # 8*8 = 64 images total
WPAD = W + 2 * PAD  # 518
S = H // 128  # 4 strips of 128 rows per image

f32 = mybir.dt.float32
f16 = mybir.dt.float16
ADD = mybir.AluOpType.add
MAX = mybir.AluOpType.max
IDENT = mybir.ActivationFunctionType.Identity

VARIANT = os.environ.get("BASS_DILATE_VARIANT", "f16")
# taps whose add runs on ACT (odd t must: fp16 4x tensor_scalar needs 4B
# alignment; odd-t slices are only 2B aligned). Then pad with even-t taps
# until ACT and DVE are balanced (~35 ACT adds per image).
N_ACT_EXTRA = int(os.environ.get("BASS_DILATE_ACT_EXTRA", "13"))


def _emit_pad_fill(nc, xpad, zt, g):
    """Zero xpad[g] (interior gets overwritten by the image afterwards)."""
    for r0 in range(0, WPAD, 128):
        r1 = min(WPAD, r0 + 128)
        nc.sync.dma_start(out=xpad[g, r0:r1, :], in_=zt[0:r1 - r0, :])


def _build_f32():
    nc = bacc.Bacc("TRN2")
    x = nc.dram_tensor("x", (IMGS_PER_CORE, H, W), f32, kind="ExternalInput")
    wt = nc.dram_tensor("weight", (KH, KW), f32, kind="ExternalInput")
    out = nc.dram_tensor("out", (IMGS_PER_CORE, H, W), f32, kind="ExternalOutput")
    NCH = 2

    with TileContext(nc) as tc:
        with (
            tc.tile_pool(name="const", bufs=1) as cpool,
            tc.tile_pool(name="dram", bufs=1, space="DRAM") as dpool,
            tc.tile_pool(name="sh", bufs=12) as shpool,
            tc.tile_pool(name="acc", bufs=2) as apool,
        ):
            w_sb = cpool.tile([128, KH * KW], f32)
            nc.sync.dma_start(
                out=w_sb[:, :],
                in_=wt[:, :].rearrange("a b -> (a b)").unsqueeze(0)
                .broadcast_to([128, KH * KW]),
            )
            zt = cpool.tile([128, WPAD], f32)
            nc.vector.memset(zt[:, :], 0.0)

            xpad = dpool.tile([IMGS_PER_CORE, WPAD, WPAD], f32)

            def fill(g):
                _emit_pad_fill(nc, xpad, zt, g)
                nc.sync.dma_start(
                    out=xpad[g, PAD:PAD + H, PAD:PAD + W], in_=x[g, :, :]
                )

            fill(0)
            fill(1)
            for g in range(IMGS_PER_CORE):
                accs = [
                    apool.tile([128, S, W], f32, tag=f"acc{c}", name=f"acc{c}_{g}")
                    for c in range(NCH)
                ]
                acc_used = [False] * NCH
                for a in range(KH):
                    sh = shpool.tile([128, S, WPAD], f32, tag="sh", name=f"sh_{g}_{a}")
                    nc.sync.dma_start(
                        out=sh[:, :, :],
                        in_=xpad[g, a:a + H, :].rearrange("(s p) w -> p s w", p=128),
                    )
                    for t in range(KW):
                        k = a * KW + t
                        c = k % NCH
                        in0 = sh[:, :, t:t + W]
                        if not acc_used[c]:
                            nc.vector.tensor_scalar(
                                out=accs[c][:, :, :], in0=in0,
                                scalar1=w_sb[:, k:k + 1], scalar2=None, op0=ADD,
                            )
                            acc_used[c] = True
                        else:
                            nc.vector.scalar_tensor_tensor(
                                out=accs[c][:, :, :], in0=in0,
                                scalar=w_sb[:, k:k + 1], in1=accs[c][:, :, :],
                                op0=ADD, op1=MAX,
                            )
                for c in range(1, NCH):
                    nc.vector.tensor_tensor(
                        out=accs[0][:, :, :], in0=accs[0][:, :, :],
                        in1=accs[c][:, :, :], op=MAX,
                    )
                nc.sync.dma_start(
                    out=out[g].rearrange("(s p) w -> p s w", p=128),
                    in_=accs[0][:, :, :],
                )
                if g + 2 < IMGS_PER_CORE:
                    fill(g + 2)
    nc.finalize()
    return nc


def _build_f16():
    nc = bacc.Bacc("TRN2")
    x = nc.dram_tensor("x", (IMGS_PER_CORE, H, W), f32, kind="ExternalInput")
    wt = nc.dram_tensor("weight", (KH, KW), f32, kind="ExternalInput")
    out = nc.dram_tensor("out", (IMGS_PER_CORE, H, W), f32, kind="ExternalOutput")
    NCH = int(os.environ.get("BASS_DILATE_NCH", "4"))
    N_DIRECT = int(os.environ.get("BASS_DILATE_DIRECT", "0"))

    # adds on ACT: all odd t (alignment), plus N_ACT_EXTRA even-t for balance
    act_taps = {(a, t) for a in range(KH) for t in range(KW) if t % 2 == 1}
    even_taps = [(a, t) for a in range(KH) for t in range(KW) if t % 2 == 0]
    step = max(1, len(even_taps) // max(1, N_ACT_EXTRA))
    for i in range(0, min(N_ACT_EXTRA, len(even_taps))):
        act_taps.add(even_taps[(i * step) % len(even_taps)])

    with TileContext(nc) as tc:
        with (
            tc.tile_pool(name="const", bufs=1) as cpool,
            tc.tile_pool(name="dram", bufs=1, space="DRAM") as dpool,
            tc.tile_pool(name="sh", bufs=12) as shpool,
            tc.tile_pool(name="tmp", bufs=8) as tpool,
            tc.tile_pool(name="acc", bufs=2) as apool,
        ):
            w_sb = cpool.tile([128, KH * KW], f32)
            nc.sync.dma_start(
                out=w_sb[:, :],
                in_=wt[:, :].rearrange("a b -> (a b)").unsqueeze(0)
                .broadcast_to([128, KH * KW]),
            )
            zt = cpool.tile([128, WPAD], f16)
            nc.vector.memset(zt[:, :], 0.0)

            xpad = dpool.tile([IMGS_PER_CORE, WPAD, WPAD], f16)

            def fill(g):
                _emit_pad_fill(nc, xpad, zt, g)
                # fp32 -> fp16 cast happens inside the SWDGE DMA
                nc.gpsimd.dma_start(
                    out=xpad[g, PAD:PAD + H, PAD:PAD + W], in_=x[g, :, :]
                )

            def load_sh_direct(sh, g, a):
                """Image g's shifted slab straight from x (SWDGE casts
                f32->f16); pad columns via DVE memset, pad rows from zt.
                No input dependencies, so the pipeline starts immediately."""
                d = a - PAD
                nc.vector.memset(sh[:, :, 0:PAD], 0.0)
                nc.vector.memset(sh[:, :, PAD + W:WPAD], 0.0)
                if d < 0:
                    nc.gpsimd.dma_start(out=sh[-d:128, 0, PAD:PAD + W],
                                        in_=x[g, 0:128 + d, :])
                    nc.gpsimd.dma_start(
                        out=sh[:, 1:S, PAD:PAD + W],
                        in_=x[g, 128 + d:(S - 1) * 128 + 128 + d, :]
                        .rearrange("(s p) w -> p s w", p=128),
                    )
                    nc.gpsimd.dma_start(out=sh[0:-d, 0, :], in_=zt[0:-d, :])
                elif d == 0:
                    nc.gpsimd.dma_start(
                        out=sh[:, :, PAD:PAD + W],
                        in_=x[g].rearrange("(s p) w -> p s w", p=128),
                    )
                else:
                    nc.gpsimd.dma_start(
                        out=sh[:, 0:S - 1, PAD:PAD + W],
                        in_=x[g, d:(S - 1) * 128 + d, :]
                        .rearrange("(s p) w -> p s w", p=128),
                    )
                    nc.gpsimd.dma_start(out=sh[0:128 - d, S - 1, PAD:PAD + W],
                                        in_=x[g, (S - 1) * 128 + d:H, :])
                    nc.gpsimd.dma_start(out=sh[128 - d:128, S - 1, :],
                                        in_=zt[0:d, :])

            for g in range(N_DIRECT, N_DIRECT + 2):
                if g < IMGS_PER_CORE:
                    fill(g)
            for g in range(IMGS_PER_CORE):
                accs = [
                    apool.tile([128, S, W], f16, tag=f"acc{c}", name=f"acc{c}_{g}")
                    for c in range(NCH)
                ]
                acc_used = [False] * NCH
                for a in range(KH):
                    sh = shpool.tile([128, S, WPAD], f16, tag="sh", name=f"sh_{g}_{a}")
                    if g < N_DIRECT:
                        load_sh_direct(sh, g, a)
                    else:
                        nc.sync.dma_start(
                            out=sh[:, :, :],
                            in_=xpad[g, a:a + H, :]
                            .rearrange("(s p) w -> p s w", p=128),
                        )
                    for t in range(KW):
                        k = a * KW + t
                        c = k % NCH
                        in0 = sh[:, :, t:t + W]
                        if not acc_used[c]:
                            dst = accs[c][:, :, :]
                        else:
                            tmp = tpool.tile([128, S, W], f16, tag="tmp",
                                             name=f"tmp_{g}_{k}")
                            dst = tmp[:, :, :]
                        if (a, t) in act_taps:
                            nc.scalar.activation(
                                out=dst, in_=in0, func=IDENT,
                                bias=w_sb[:, k:k + 1], scale=1.0,
                            )
                        else:
                            nc.vector.tensor_scalar(
                                out=dst, in0=in0,
                                scalar1=w_sb[:, k:k + 1], scalar2=None, op0=ADD,
                            )
                        if acc_used[c]:
                            nc.vector.tensor_tensor(
                                out=accs[c][:, :, :], in0=accs[c][:, :, :],
                                in1=dst, op=MAX,
                            )
                        acc_used[c] = True
                for c in range(1, NCH):
                    nc.vector.tensor_tensor(
                        out=accs[0][:, :, :], in0=accs[0][:, :, :],
                        in1=accs[c][:, :, :], op=MAX,
                    )
                nc.gpsimd.dma_start(
                    out=out[g].rearrange("(s p) w -> p s w", p=128),
                    in_=accs[0][:, :, :],
                )
                if g + N_DIRECT + 2 < IMGS_PER_CORE:
                    fill(g + N_DIRECT + 2)
    nc.finalize()
    return nc


_NC_CACHE = {}


def _get_nc(variant=None):
    variant = variant or VARIANT
    if variant not in _NC_CACHE:
        _NC_CACHE[variant] = _build_f16() if variant == "f16" else _build_f32()
    return _NC_CACHE[variant]


def _run(x, weight, trace=False, variant=None, trace_kwargs=None):
    x = np.ascontiguousarray(x, dtype=np.float32)
    weight = np.ascontiguousarray(weight, dtype=np.float32)
    B, C, Hx, Wx = x.shape
    xs = x.reshape(B * C, Hx, Wx)
    per = (B * C) // N_CORES
    in_maps = [
        {"x": np.ascontiguousarray(xs[i * per:(i + 1) * per]), "weight": weight}
        for i in range(N_CORES)
    ]
    nc = _get_nc(variant)
    res = run_bass_kernel_spmd(
        nc, in_maps, list(range(N_CORES)),
        trace=trace, trace_cores=[0] if trace else None,
        **(trace_kwargs or {}),
    )
    outs = np.concatenate([res.results[i]["out"] for i in range(N_CORES)], axis=0)
    return outs.reshape(B, C, Hx, Wx), res


def kernel(x, weight):
    out, _ = _run(x, weight)
    return out
